# revision 2
# baseline (speedup 1.0000x reference)
"""Trainium2 Bass kernel for nn_ClusteringModel (vq_codebook).

Math (reference, R=2, Q=1, c=1, beta=3, Tc=1, Twta=0.1, phi=1.5):
  a        = attn / sum(attn)                       (normalized attention)
  wdist    = sum_d a_d * (x_bd - w_cd)^2            [B, C]
  r        = sqrt(wdist)
  H        = exp(-r)
  E1       = exp(-3r) (masked -> 0);  s1 = sum_c E1
  competed = (E1 * H / s1) = exp(-4r)/s1            (masked -> 0)
  E2       = exp(10*competed) (masked -> 0);  s2 = sum_c E2
  y        = 1.5 * (E2 * exp(-4r) / (s1*s2)) @ w_assoc

Key algebraic transform: with raw u = attn and S = sum(u),
  wdist*S = sum_d u x^2 - 2 sum_d u x w + sum_d u w^2
so the [B,C,D] distance tensor is never materialized; the cross term is a
matmul and the w^2 term is a ones-row matmul (PE broadcasts it over B).
Masking is injected additively (+BIG into wdist for the first softmax, and
-BIG into the WTA logits) so no [B,C] mask multiplies are needed.

Sharding: data-parallel over batch. 8 cores x 128 rows each; codebook
(w_dist), attn, w_assoc, mask replicated. Host does layout prep only
(transposes / slicing); all model FLOPs run on device.
"""

import sys

if "/opt/trn_rl_repo" not in sys.path:
    sys.path.insert(0, "/opt/trn_rl_repo")

import numpy as np

B, C, D = 1024, 512, 256
N_CORES = 8
BS = B // N_CORES          # 128 batch rows per core
KC = D // 128              # 2 contraction chunks of 128
BIG = 1.0e30
EPS_RAW = 0.01             # tiny additive guard (raw, pre-1/S units) so
                           # sqrt never sees a negative from fp cancellation

_CACHE = {}


def _build(matmul_dt_name="float32r"):
    import concourse.bacc as bacc
    import concourse.mybir as mybir
    import concourse.tile as tile

    mdt = getattr(mybir.dt, matmul_dt_name)
    f32 = mybir.dt.float32
    i32 = mybir.dt.int32
    AF = mybir.ActivationFunctionType
    OP = mybir.AluOpType

    nc = bacc.Bacc("TRN2", target_bir_lowering=False)

    xT = nc.dram_tensor("xT", [D, BS], f32, kind="ExternalInput")
    wT = nc.dram_tensor("wT", [D, C], mdt, kind="ExternalInput")
    u = nc.dram_tensor("u", [D], f32, kind="ExternalInput")
    mask = nc.dram_tensor("mask", [C], i32, kind="ExternalInput")
    waT = nc.dram_tensor("waT", [2, C], f32, kind="ExternalInput")
    y = nc.dram_tensor("y", [BS, 2], f32, kind="ExternalOutput")

    with tile.TileContext(nc) as tc:
        with (
            tc.tile_pool(name="data", bufs=1) as dp,
            tc.tile_pool(name="psum", bufs=1, space="PSUM") as pp,
        ):
            # ---------- loads ----------
            xT_sb = dp.tile([128, KC, BS], f32, tag="xT_sb")
            nc.sync.dma_start(out=xT_sb, in_=xT.rearrange("(k p) b -> p k b", p=128))

            wT_sb = dp.tile([128, KC, C], mdt, tag="wT_sb")
            nc.sync.dma_start(out=wT_sb, in_=wT.rearrange("(k p) c -> p k c", p=128))

            u_sb = dp.tile([128, KC], f32, tag="u_sb")
            nc.sync.dma_start(out=u_sb, in_=u.rearrange("(k p) -> p k", p=128))

            mask_sb = dp.tile([1, C], i32, tag="mask_sb")
            nc.sync.dma_start(out=mask_sb, in_=mask.rearrange("(a c) -> a c", a=1))

            # broadcast w_assoc.T rows across all 128 partitions
            import concourse.bass as bass

            wa_sb = dp.tile([128, 2, C], f32, tag="wa_sb")
            wa_bcast = bass.AP(tensor=waT, offset=0, ap=[[0, 128], [C, 2], [1, C]])
            nc.sync.dma_start(out=wa_sb, in_=wa_bcast)

            # ---------- small prep ----------
            ones = dp.tile([128, 128], mdt, tag="ones")
            nc.vector.memset(ones, 1.0)

            su = dp.tile([128, KC], f32, tag="su")       # sqrt(u)
            nc.scalar.activation(su, u_sb, AF.Sqrt)
            un2 = dp.tile([128, KC], f32, tag="un2")     # -2u
            nc.scalar.mul(un2, u_sb, -2.0)

            # mask rows:  mrow_pos = BIG*(1-m)  (>=0),  mrow_neg = -BIG*(1-m)
            mrow_pos = dp.tile([1, C], mdt, tag="mrow_pos")
            nc.vector.tensor_scalar(
                out=mrow_pos, in0=mask_sb, scalar1=-BIG, scalar2=BIG,
                op0=OP.mult, op1=OP.add,
            )
            mrow_neg = dp.tile([1, C], f32, tag="mrow_neg")
            nc.vector.tensor_scalar(
                out=mrow_neg, in0=mask_sb, scalar1=BIG, scalar2=-BIG,
                op0=OP.mult, op1=OP.add,
            )
            # broadcast -BIG*(1-m) to all partitions for the WTA logit add
            mneg_bc = dp.tile([128, C], f32, tag="mneg_bc")
            nc.gpsimd.partition_broadcast(mneg_bc, mrow_neg)

            # xsq = xT^2  (flat view over chunks)
            xsq = dp.tile([128, KC, BS], f32, tag="xsq")
            nc.vector.tensor_mul(
                xsq.rearrange("p k b -> p (k b)"),
                xT_sb.rearrange("p k b -> p (k b)"),
                xT_sb.rearrange("p k b -> p (k b)"),
            )

            # xu2 = -2u * xT   (per-partition scalar: u lives on partitions here)
            xu2 = dp.tile([128, KC, BS], mdt, tag="xu2")
            for k in range(KC):
                nc.vector.tensor_scalar_mul(xu2[:, k, :], xT_sb[:, k, :], un2[:, k : k + 1])

            # R2 = u * wT^2 = (sqrt(u) * wT)^2   (ACT Square with per-part scale)
            R2 = dp.tile([128, KC, C], mdt, tag="R2")
            for k in range(KC):
                nc.scalar.activation(
                    R2[:, k, :], wT_sb[:, k, :], AF.Square, scale=su[:, k : k + 1]
                )

            # ---------- PE ----------
            psum_t1 = pp.tile([128, 1], f32, tag="psum_t1")
            psum_S = pp.tile([128, 1], f32, tag="psum_S")
            psum_main = pp.tile([128, C], f32, tag="psum_main")

            # t1[b] = sum_d u x^2   (N=1, nearly free)
            for k in range(KC):
                nc.tensor.matmul(
                    psum_t1, lhsT=xsq[:, k, :], rhs=u_sb[:, k : k + 1],
                    start=(k == 0), stop=(k == KC - 1),
                )
            # S = sum_d u   (broadcast to every partition via ones lhsT)
            ones_f32 = dp.tile([128, 128], f32, tag="ones_f32")
            nc.vector.memset(ones_f32, 1.0)
            for k in range(KC):
                nc.tensor.matmul(
                    psum_S, lhsT=ones_f32, rhs=u_sb[:, k : k + 1],
                    start=(k == 0), stop=(k == KC - 1),
                )
            # main accumulation:
            #   psum_main = -2 sum_d u x w  +  sum_d u w^2  +  BIG*(1-m)
            for k in range(KC):
                nc.tensor.matmul(
                    psum_main, lhsT=xu2[:, k, :], rhs=wT_sb[:, k, :],
                    start=(k == 0), stop=False,
                )
            for k in range(KC):
                nc.tensor.matmul(psum_main, lhsT=ones, rhs=R2[:, k, :], start=False, stop=False)
            nc.tensor.matmul(psum_main, lhsT=ones[0:1, :], rhs=mrow_pos, start=False, stop=True)

            # ---------- epilogue ----------
            invS = dp.tile([128, 1], f32, tag="invS")
            nc.vector.reciprocal(invS, psum_S)

            t1 = dp.tile([128, 1], f32, tag="t1")
            nc.scalar.activation(t1, psum_t1, AF.Copy, bias=EPS_RAW)

            # wdist = max(0, (psum + t1 + eps)) / S
            wdist = dp.tile([128, C], f32, tag="wdist")
            nc.vector.tensor_scalar(
                out=wdist, in0=psum_main, scalar1=t1, scalar2=invS,
                op0=OP.add, op1=OP.mult,
            )
            wdist2 = dp.tile([128, C], f32, tag="wdist2")
            nc.gpsimd.tensor_scalar_max(wdist2, wdist, 0.0)

            r = dp.tile([128, C], f32, tag="r")
            nc.scalar.activation(r, wdist2, AF.Sqrt)

            # E1 = exp(-3r), s1 = sum_c E1   (masked entries are exp(-huge)=0)
            E1 = dp.tile([128, C], f32, tag="E1")
            s1 = dp.tile([128, 1], f32, tag="s1")
            nc.scalar.activation(E1, r, AF.Exp, scale=-3.0, accum_out=s1)

            # v = E1*H = exp(-4r)
            v = dp.tile([128, C], f32, tag="v")
            nc.scalar.activation(v, r, AF.Exp, scale=-4.0)

            r1 = dp.tile([128, 1], f32, tag="r1")
            nc.vector.reciprocal(r1, s1)
            r110 = dp.tile([128, 1], f32, tag="r110")    # 10/s1
            nc.scalar.mul(r110, r1, 10.0)

            # wta_pre = v + (-BIG*(1-m));  E2 = exp(10/s1 * wta_pre), s2 = sum
            wta = dp.tile([128, C], f32, tag="wta")
            nc.vector.tensor_add(wta, v, mneg_bc)
            E2 = dp.tile([128, C], f32, tag="E2")
            s2 = dp.tile([128, 1], f32, tag="s2")
            nc.scalar.activation(E2, wta, AF.Exp, scale=r110, accum_out=s2)

            r2 = dp.tile([128, 1], f32, tag="r2")
            nc.vector.reciprocal(r2, s2)

            # u2 = E2 * v;  y_j = sum_c u2 * waT[j]  (fused mult+reduce)
            u2 = dp.tile([128, C], f32, tag="u2")
            nc.vector.tensor_mul(u2, E2, v)

            yt = dp.tile([128, 2], f32, tag="yt")
            scr0 = dp.tile([128, C], f32, tag="scr0")
            scr1 = dp.tile([128, C], f32, tag="scr1")
            nc.vector.scalar_tensor_tensor(
                out=scr0, in0=u2, scalar=1.0, in1=wa_sb[:, 0, :],
                op0=OP.mult, op1=OP.mult, accum_out=yt[:, 0:1],
            )
            nc.vector.scalar_tensor_tensor(
                out=scr1, in0=u2, scalar=1.0, in1=wa_sb[:, 1, :],
                op0=OP.mult, op1=OP.mult, accum_out=yt[:, 1:2],
            )

            # y = yt * (1.5 * r1 * r2)
            rfin = dp.tile([128, 1], f32, tag="rfin")
            nc.vector.tensor_scalar(
                out=rfin, in0=r1, scalar1=r2, scalar2=1.5, op0=OP.mult, op1=OP.mult
            )
            y_sb = dp.tile([128, 2], f32, tag="y_sb")
            nc.vector.tensor_scalar_mul(y_sb, yt, rfin)

            nc.sync.dma_start(out=y[:, :], in_=y_sb)

    nc.compile()
    return nc


def _get_nc(matmul_dt_name="float32r"):
    if matmul_dt_name not in _CACHE:
        _CACHE[matmul_dt_name] = _build(matmul_dt_name)
    return _CACHE[matmul_dt_name]


def kernel(inp, w_dist, attn, w_assoc, mask, _trace=False, _tmpdir=None,
           _matmul_dt="float32r"):
    from concourse.bass_utils import run_bass_kernel_spmd

    inp = np.asarray(inp, dtype=np.float32)
    w_dist = np.asarray(w_dist, dtype=np.float32)
    attn = np.asarray(attn, dtype=np.float32)
    w_assoc = np.asarray(w_assoc, dtype=np.float32)
    mask = np.asarray(mask, dtype=np.int32)

    # host-side layout prep (no model FLOPs): transposes + batch sharding
    xT_full = np.ascontiguousarray(inp.T)          # [D, B]
    wT = np.ascontiguousarray(w_dist.T)            # [D, C]
    waT = np.ascontiguousarray(w_assoc.T)          # [2, C]

    nc = _get_nc(_matmul_dt)

    in_maps = []
    for i in range(N_CORES):
        in_maps.append(
            {
                "xT": np.ascontiguousarray(xT_full[:, i * BS : (i + 1) * BS]),
                "wT": wT,
                "u": attn,
                "mask": mask,
                "waT": waT,
            }
        )

    kw = {}
    if _trace:
        kw["trace"] = True
        if _tmpdir:
            kw["tmpdir"] = _tmpdir
    res = run_bass_kernel_spmd(nc, in_maps, core_ids=list(range(N_CORES)), **kw)
    out = np.concatenate([res.results[i]["y"] for i in range(N_CORES)], axis=0)
    if _trace:
        return out.astype(np.float32), res
    return out.astype(np.float32)


# revision 5
# speedup vs baseline: 1.0820x; 1.0820x over previous
"""Trainium2 Bass kernel for nn_ClusteringModel (vq_codebook).

Math (reference, R=2, Q=1, c=1, beta=3, Tc=1, Twta=0.1, phi=1.5):
  a        = attn / sum(attn)                       (normalized attention)
  wdist    = sum_d a_d * (x_bd - w_cd)^2            [B, C]
  r        = sqrt(wdist)
  H        = exp(-r)
  E1       = exp(-3r) (masked -> 0);  s1 = sum_c E1
  competed = (E1 * H / s1) = exp(-4r)/s1            (masked -> 0)
  E2       = exp(10*competed) (masked -> 0);  s2 = sum_c E2
  y        = 1.5 * (E2 * exp(-4r) / (s1*s2)) @ w_assoc

Key algebraic transform: with raw u = attn and S = sum(u),
  wdist*S = sum_d u x^2 - 2 sum_d u x w + sum_d u w^2
so the [B,C,D] distance tensor is never materialized; the cross term is a
matmul and the w^2 term is a ones-row matmul (PE broadcasts it over B).
Masking is injected additively (+BIG into wdist for the first softmax, and
-BIG into the WTA logits) so no [B,C] mask multiplies are needed.

Sharding: data-parallel over batch. 8 cores x 128 rows each; codebook
(w_dist), attn, w_assoc, mask replicated. Host does layout prep only
(transposes / slicing); all model FLOPs run on device.
"""

import sys

if "/opt/trn_rl_repo" not in sys.path:
    sys.path.insert(0, "/opt/trn_rl_repo")

import numpy as np

B, C, D = 1024, 512, 256
N_CORES = 8
BS = B // N_CORES          # 128 batch rows per core
KC = D // 128              # 2 contraction chunks of 128
BIG = 1.0e30
EPS_RAW = 0.01             # tiny additive guard (raw, pre-1/S units) so
                           # sqrt never sees a negative from fp cancellation

_CACHE = {}


def _build(matmul_dt_name="float32r"):
    import concourse.bacc as bacc
    import concourse.mybir as mybir
    import concourse.tile as tile

    import dataclasses

    mdt = getattr(mybir.dt, matmul_dt_name)
    f32 = mybir.dt.float32
    i32 = mybir.dt.int32
    AF = mybir.ActivationFunctionType
    OP = mybir.AluOpType

    def mm_view(ap):
        """Bitcast an f32 AP to the matmul dtype (same bytes, same tensor)."""
        if mdt == f32:
            return ap
        return dataclasses.replace(
            ap, tensor=dataclasses.replace(ap.tensor, dtype=mdt)
        )

    nc = bacc.Bacc("TRN2", target_bir_lowering=False)

    xT = nc.dram_tensor("xT", [D, BS], f32, kind="ExternalInput")
    wT = nc.dram_tensor("wT", [D, C], mdt, kind="ExternalInput")
    u = nc.dram_tensor("u", [D], f32, kind="ExternalInput")
    mask = nc.dram_tensor("mask", [C], i32, kind="ExternalInput")
    waT = nc.dram_tensor("waT", [2, C], f32, kind="ExternalInput")
    y = nc.dram_tensor("y", [BS, 2], f32, kind="ExternalOutput")

    with tile.TileContext(nc) as tc:
        with (
            tc.tile_pool(name="data", bufs=1) as dp,
            tc.tile_pool(name="psum", bufs=1, space="PSUM") as pp,
        ):
            # ---------- loads ----------
            xT_sb = dp.tile([128, KC, BS], f32, tag="xT_sb")
            nc.sync.dma_start(out=xT_sb, in_=xT.rearrange("(k p) b -> p k b", p=128))

            wT_sb = dp.tile([128, KC, C], mdt, tag="wT_sb")
            nc.sync.dma_start(out=wT_sb, in_=wT.rearrange("(k p) c -> p k c", p=128))

            u_sb = dp.tile([128, KC], f32, tag="u_sb")
            nc.sync.dma_start(out=u_sb, in_=u.rearrange("(k p) -> p k", p=128))

            mask_sb = dp.tile([1, C], i32, tag="mask_sb")
            nc.sync.dma_start(out=mask_sb, in_=mask.rearrange("(a c) -> a c", a=1))

            # broadcast w_assoc.T rows across all 128 partitions
            import concourse.bass as bass

            wa_sb = dp.tile([128, 2, C], f32, tag="wa_sb")
            wa_bcast = bass.AP(tensor=waT, offset=0, ap=[[0, 128], [C, 2], [1, C]])
            nc.sync.dma_start(out=wa_sb, in_=wa_bcast)

            # ---------- small prep ----------
            ones_f32 = dp.tile([128, 128], f32, tag="ones_f32")
            nc.vector.memset(ones_f32, 1.0)
            if mdt == f32:
                ones = ones_f32
            else:
                ones = dp.tile([128, 128], mdt, tag="ones")
                nc.scalar.copy(ones, ones_f32)

            su = dp.tile([128, KC], f32, tag="su")       # sqrt(u)
            nc.scalar.activation(su, u_sb, AF.Sqrt)
            un2 = dp.tile([128, KC], f32, tag="un2")     # -2u
            nc.scalar.mul(un2, u_sb, -2.0)

            # mask rows:  mrow_pos = BIG*(1-m)  (>=0),  mrow_neg = -BIG*(1-m)
            mrow_pos = dp.tile([1, C], mdt, tag="mrow_pos")
            nc.vector.tensor_scalar(
                out=mrow_pos, in0=mask_sb, scalar1=-BIG, scalar2=BIG,
                op0=OP.mult, op1=OP.add,
            )
            mrow_neg = dp.tile([1, C], f32, tag="mrow_neg")
            nc.vector.tensor_scalar(
                out=mrow_neg, in0=mask_sb, scalar1=BIG, scalar2=-BIG,
                op0=OP.mult, op1=OP.add,
            )
            # broadcast -BIG*(1-m) to all partitions for the WTA logit add
            mneg_bc = dp.tile([128, C], f32, tag="mneg_bc")
            nc.gpsimd.partition_broadcast(mneg_bc, mrow_neg)

            # xsq = xT^2  (flat view over chunks)
            xsq = dp.tile([128, KC, BS], f32, tag="xsq")
            nc.vector.tensor_mul(
                xsq.rearrange("p k b -> p (k b)"),
                xT_sb.rearrange("p k b -> p (k b)"),
                xT_sb.rearrange("p k b -> p (k b)"),
            )

            # xu2 = -2u * xT   (per-partition scalar: u lives on partitions here)
            xu2 = dp.tile([128, KC, BS], mdt, tag="xu2")
            for k in range(KC):
                nc.vector.tensor_scalar_mul(xu2[:, k, :], xT_sb[:, k, :], un2[:, k : k + 1])

            # R2 = u * wT^2 = (sqrt(u) * wT)^2   (ACT Square with per-part scale)
            R2 = dp.tile([128, KC, C], mdt, tag="R2")
            for k in range(KC):
                nc.scalar.activation(
                    R2[:, k, :], wT_sb[:, k, :], AF.Square, scale=su[:, k : k + 1]
                )

            # ---------- PE ----------
            psum_t1 = pp.tile([128, 1], f32, tag="psum_t1")
            psum_S = pp.tile([128, 1], f32, tag="psum_S")
            psum_main = pp.tile([128, C], f32, tag="psum_main")

            # t1[b] = sum_d u x^2   (N=1, nearly free)
            for k in range(KC):
                nc.tensor.matmul(
                    psum_t1, lhsT=xsq[:, k, :], rhs=u_sb[:, k : k + 1],
                    start=(k == 0), stop=(k == KC - 1),
                )
            # S = sum_d u   (broadcast to every partition via ones lhsT)
            for k in range(KC):
                nc.tensor.matmul(
                    psum_S, lhsT=ones_f32, rhs=u_sb[:, k : k + 1],
                    start=(k == 0), stop=(k == KC - 1),
                )
            # main accumulation:
            #   psum_main = -2 sum_d u x w  +  sum_d u w^2  +  BIG*(1-m)
            for k in range(KC):
                nc.tensor.matmul(
                    psum_main, lhsT=xu2[:, k, :], rhs=wT_sb[:, k, :],
                    start=(k == 0), stop=False,
                )
            for k in range(KC):
                nc.tensor.matmul(psum_main, lhsT=ones, rhs=R2[:, k, :], start=False, stop=False)
            nc.tensor.matmul(psum_main, lhsT=ones[0:1, :], rhs=mrow_pos, start=False, stop=True)

            # ---------- epilogue ----------
            invS = dp.tile([128, 1], f32, tag="invS")
            nc.vector.reciprocal(invS, psum_S)

            t1 = dp.tile([128, 1], f32, tag="t1")
            nc.scalar.activation(t1, psum_t1, AF.Copy, bias=EPS_RAW)

            # wdist = max(0, (psum + t1 + eps)) / S
            wdist = dp.tile([128, C], f32, tag="wdist")
            nc.vector.tensor_scalar(
                out=wdist, in0=psum_main, scalar1=t1, scalar2=invS,
                op0=OP.add, op1=OP.mult,
            )
            wdist2 = dp.tile([128, C], f32, tag="wdist2")
            nc.gpsimd.tensor_scalar_max(wdist2, wdist, 0.0)

            r = dp.tile([128, C], f32, tag="r")
            nc.scalar.activation(r, wdist2, AF.Sqrt)

            # E1 = exp(-3r), s1 = sum_c E1   (masked entries are exp(-huge)=0)
            E1 = dp.tile([128, C], f32, tag="E1")
            s1 = dp.tile([128, 1], f32, tag="s1")
            nc.scalar.activation(E1, r, AF.Exp, scale=-3.0, accum_out=s1)

            # v = E1*H = exp(-4r)
            v = dp.tile([128, C], f32, tag="v")
            nc.scalar.activation(v, r, AF.Exp, scale=-4.0)

            r1 = dp.tile([128, 1], f32, tag="r1")
            nc.vector.reciprocal(r1, s1)
            r110 = dp.tile([128, 1], f32, tag="r110")    # 10/s1
            nc.scalar.mul(r110, r1, 10.0)

            # wta_pre = v + (-BIG*(1-m));  E2 = exp(10/s1 * wta_pre), s2 = sum
            wta = dp.tile([128, C], f32, tag="wta")
            nc.vector.tensor_add(wta, v, mneg_bc)
            E2 = dp.tile([128, C], f32, tag="E2")
            s2 = dp.tile([128, 1], f32, tag="s2")
            nc.scalar.activation(E2, wta, AF.Exp, scale=r110, accum_out=s2)

            r2 = dp.tile([128, 1], f32, tag="r2")
            nc.vector.reciprocal(r2, s2)

            # u2 = E2 * v;  y_j = sum_c u2 * waT[j]  (fused mult+reduce)
            u2 = dp.tile([128, C], f32, tag="u2")
            nc.vector.tensor_mul(u2, E2, v)

            yt = dp.tile([128, 2], f32, tag="yt")
            scr0 = dp.tile([128, C], f32, tag="scr0")
            scr1 = dp.tile([128, C], f32, tag="scr1")
            nc.vector.scalar_tensor_tensor(
                out=scr0, in0=u2, scalar=1.0, in1=wa_sb[:, 0, :],
                op0=OP.mult, op1=OP.mult, accum_out=yt[:, 0:1],
            )
            nc.vector.scalar_tensor_tensor(
                out=scr1, in0=u2, scalar=1.0, in1=wa_sb[:, 1, :],
                op0=OP.mult, op1=OP.mult, accum_out=yt[:, 1:2],
            )

            # y = yt * (1.5 * r1 * r2)
            rfin = dp.tile([128, 1], f32, tag="rfin")
            nc.vector.tensor_scalar(
                out=rfin, in0=r1, scalar1=r2, scalar2=1.5, op0=OP.mult, op1=OP.mult
            )
            y_sb = dp.tile([128, 2], f32, tag="y_sb")
            nc.vector.tensor_scalar_mul(y_sb, yt, rfin)

            nc.sync.dma_start(out=y[:, :], in_=y_sb)

    nc.compile()
    return nc


def _get_nc(matmul_dt_name="float32r"):
    if matmul_dt_name not in _CACHE:
        _CACHE[matmul_dt_name] = _build(matmul_dt_name)
    return _CACHE[matmul_dt_name]


def kernel(inp, w_dist, attn, w_assoc, mask, _trace=False, _tmpdir=None,
           _matmul_dt="float32r"):
    from concourse.bass_utils import run_bass_kernel_spmd

    inp = np.asarray(inp, dtype=np.float32)
    w_dist = np.asarray(w_dist, dtype=np.float32)
    attn = np.asarray(attn, dtype=np.float32)
    w_assoc = np.asarray(w_assoc, dtype=np.float32)
    mask = np.asarray(mask, dtype=np.int32)

    # host-side layout prep (no model FLOPs): transposes + batch sharding
    xT_full = np.ascontiguousarray(inp.T)          # [D, B]
    wT = np.ascontiguousarray(w_dist.T)            # [D, C]
    waT = np.ascontiguousarray(w_assoc.T)          # [2, C]

    nc = _get_nc(_matmul_dt)

    in_maps = []
    for i in range(N_CORES):
        in_maps.append(
            {
                "xT": np.ascontiguousarray(xT_full[:, i * BS : (i + 1) * BS]),
                "wT": wT,
                "u": attn,
                "mask": mask,
                "waT": waT,
            }
        )

    kw = {}
    if _trace:
        kw["trace"] = True
        if _tmpdir:
            kw["tmpdir"] = _tmpdir
    res = run_bass_kernel_spmd(nc, in_maps, core_ids=list(range(N_CORES)), **kw)
    out = np.concatenate([res.results[i]["y"] for i in range(N_CORES)], axis=0)
    if _trace:
        return out.astype(np.float32), res
    return out.astype(np.float32)


# revision 8
# speedup vs baseline: 1.3866x; 1.2815x over previous
"""Trainium2 Bass kernel for nn_ClusteringModel (vq_codebook).

Math (reference constants R=2, Q=1, c=1, beta=3, Tc=1, Twta=0.1, phi=1.5):
  a        = attn / S,  S = sum(attn)
  wdist    = sum_d a_d (x_bd - w_cd)^2                [B, C]
  r        = sqrt(wdist);  H = exp(-r)
  p_comp   = softmax_c(-3r over recruited);  competed = p_comp * H * m
  p_wta    = softmax_c(competed/0.1 over recruited)
  y        = 1.5 * (p_wta * competed) @ w_assoc

Algebra used by the kernel (u = raw attn):
  wdist*S  = sum_d u x^2 - 2 sum_d u x w + sum_d u w^2
  E1 = exp(-3r) -> s1;  v = exp(-4r) = E1*H;  competed = v / s1
  E2 = exp(10*competed) masked -> s2
  y  = 1.5/(s1*s2) * (E2*v) @ w_assoc

The [B,C,D] distance tensor is never materialized: the cross term is one
K=256 matmul, sum_d u w^2 rides the same PSUM accumulation via a ones-block
lhsT, and masking enters additively (+BIG into wdist; -BIG into WTA logits
via a K=1 ones-row matmul that also acts as a partition-broadcast).

Sharding: data-parallel over batch (8 cores x 128 rows); w_dist / attn /
w_assoc / mask replicated. Host does layout prep only (transpose, slicing,
concat); every model FLOP runs on device.
"""

import sys

if "/opt/trn_rl_repo" not in sys.path:
    sys.path.insert(0, "/opt/trn_rl_repo")

import numpy as np

B, C, D = 1024, 512, 256
N_CORES = 8
BS = B // N_CORES          # 128 batch rows per core
KC = D // 128              # 2 contraction chunks of 128
BIG = 1.0e30
EPS_RAW = 0.01             # additive guard (pre-1/S units): fp32 error on the
                           # ~|300| psum accumulation is < 1e-3, so +0.01 keeps
                           # the sqrt argument strictly positive

_CACHE = {}


def _build(matmul_dt_name="float32r"):
    import dataclasses

    import concourse.bacc as bacc
    import concourse.mybir as mybir
    import concourse.tile as tile

    mdt = getattr(mybir.dt, matmul_dt_name)
    f32 = mybir.dt.float32
    i32 = mybir.dt.int32
    AF = mybir.ActivationFunctionType
    OP = mybir.AluOpType

    def dtv(ap, dt):
        """Bitcast view of an AP with a different dtype (same bytes)."""
        if ap.tensor.dtype == dt:
            return ap
        return dataclasses.replace(ap, tensor=dataclasses.replace(ap.tensor, dtype=dt))

    nc = bacc.Bacc("TRN2", target_bir_lowering=False)

    # big: [xT | wT] concatenated on the free dim -> one DMA
    big = nc.dram_tensor("big", [D, BS + C], mdt, kind="ExternalInput")
    # smalls: [mask bits (C) | w_assoc.T flat (2C)] -> one DMA
    smalls = nc.dram_tensor("smalls", [1, 3 * C], mdt, kind="ExternalInput")
    u = nc.dram_tensor("u", [D], f32, kind="ExternalInput")
    y = nc.dram_tensor("y", [BS, 2], f32, kind="ExternalOutput")

    with tile.TileContext(nc) as tc:
        with (
            tc.tile_pool(name="data", bufs=1) as dp,
            tc.tile_pool(name="psum", bufs=1, space="PSUM") as pp,
        ):
            # ---------- constants + ACT table warmup (overlaps the DMAs) ----------
            ones_f32 = dp.tile([128, 128], f32, tag="ones_f32")
            nc.vector.memset(ones_f32, 1.0)
            warm = dp.tile([1, 4], f32, tag="warm")
            nc.scalar.activation(warm[:, 0:1], ones_f32[0:1, 0:1], AF.Sqrt)
            nc.scalar.activation(warm[:, 1:2], ones_f32[0:1, 0:1], AF.Square)
            nc.scalar.activation(warm[:, 2:3], ones_f32[0:1, 0:1], AF.Exp)

            ones = dp.tile([128, 128], mdt, tag="ones")
            nc.scalar.copy(ones, ones_f32)

            # ---------- loads ----------
            big_sb = dp.tile([128, KC, BS + C], mdt, tag="big_sb")
            nc.sync.dma_start(out=big_sb, in_=big.rearrange("(k p) n -> p k n", p=128))
            xT_sb = big_sb[:, :, 0:BS]            # [128, KC, 128]  (d, k, b)
            wT_sb = big_sb[:, :, BS : BS + C]     # [128, KC, 512]  (d, k, c)

            sm_sb = dp.tile([1, 3 * C], mdt, tag="sm_sb")
            nc.sync.dma_start(out=sm_sb, in_=smalls[:, :])
            mask_f = sm_sb[:, 0:C]                # [1, 512] mask as 0.0/1.0
            wa_row = sm_sb[:, C : 3 * C]          # [1, 1024] = w_assoc.T flat

            u_sb = dp.tile([128, KC], f32, tag="u_sb")
            nc.sync.dma_start(out=u_sb, in_=u.rearrange("(k p) -> p k", p=128))

            # ---------- small prep ----------
            su = dp.tile([128, KC], f32, tag="su")       # sqrt(u)
            nc.scalar.activation(su, u_sb, AF.Sqrt)
            un2 = dp.tile([128, KC], f32, tag="un2")     # -2u
            nc.scalar.mul(un2, u_sb, -2.0)

            # mrow = BIG*(1-m)  (0 recruited / +BIG masked)
            mrow = dp.tile([1, C], mdt, tag="mrow")
            nc.vector.tensor_scalar(
                out=mrow, in0=mask_f, scalar1=-BIG, scalar2=BIG,
                op0=OP.mult, op1=OP.add,
            )

            # xsq = xT^2 ; xu2 = -2u * xT
            xsq = dp.tile([128, KC, BS], f32, tag="xsq")
            nc.vector.tensor_mul(xsq, dtv(xT_sb, f32), dtv(xT_sb, f32))
            xu2 = dp.tile([128, KC, BS], mdt, tag="xu2")
            for k in range(KC):
                nc.vector.tensor_scalar_mul(xu2[:, k, :], xT_sb[:, k, :], un2[:, k : k + 1])

            # R2 = u * wT^2 = (sqrt(u) * wT)^2
            R2 = dp.tile([128, KC, C], mdt, tag="R2")
            for k in range(KC):
                nc.scalar.activation(
                    R2[:, k, :], wT_sb[:, k, :], AF.Square, scale=su[:, k : k + 1]
                )

            # ---------- PE ----------
            psum_wa = pp.tile([128, 2, C], f32, tag="psum_wa")    # w_assoc bcast
            psum_mask = pp.tile([128, C], f32, tag="psum_mask")   # mrow bcast
            psum_t1 = pp.tile([128, 1], f32, tag="psum_t1")
            psum_S = pp.tile([128, 1], f32, tag="psum_S")
            psum_main = pp.tile([128, C], f32, tag="psum_main")

            ones_row = ones[0:1, :]
            # broadcasts via K=1 matmuls (PE fans a [1,N] row out to 128 rows)
            for j in range(2):
                nc.tensor.matmul(
                    psum_wa[:, j, :], lhsT=ones_row, rhs=wa_row[:, j * C : (j + 1) * C],
                    start=True, stop=True,
                )
            nc.tensor.matmul(psum_mask, lhsT=ones_row, rhs=mrow, start=True, stop=True)

            # t1[b] = sum_d u x^2 ;  S = sum_d u  (per-partition via ones lhsT)
            for k in range(KC):
                nc.tensor.matmul(
                    psum_t1, lhsT=xsq[:, k, :], rhs=u_sb[:, k : k + 1],
                    start=(k == 0), stop=(k == KC - 1),
                )
            for k in range(KC):
                nc.tensor.matmul(
                    psum_S, lhsT=ones_f32, rhs=u_sb[:, k : k + 1],
                    start=(k == 0), stop=(k == KC - 1),
                )
            # psum_main = -2 sum_d u x w + sum_d u w^2 + BIG*(1-m)
            for k in range(KC):
                nc.tensor.matmul(
                    psum_main, lhsT=xu2[:, k, :], rhs=wT_sb[:, k, :],
                    start=(k == 0), stop=False,
                )
            for k in range(KC):
                nc.tensor.matmul(psum_main, lhsT=ones, rhs=R2[:, k, :], start=False, stop=False)
            nc.tensor.matmul(psum_main, lhsT=ones_row, rhs=mrow, start=False, stop=True)

            # ---------- epilogue ----------
            invS = dp.tile([128, 1], f32, tag="invS")
            nc.vector.reciprocal(invS, psum_S)
            t1e = dp.tile([128, 1], f32, tag="t1e")
            nc.vector.tensor_scalar_add(t1e, psum_t1, EPS_RAW)

            # wdist = (psum + t1 + eps) / S   (>0 by construction)
            wdist = dp.tile([128, C], f32, tag="wdist")
            nc.vector.tensor_scalar(
                out=wdist, in0=psum_main, scalar1=t1e, scalar2=invS,
                op0=OP.add, op1=OP.mult,
            )
            r = dp.tile([128, C], f32, tag="r")
            nc.scalar.activation(r, wdist, AF.Sqrt)

            # E1 = exp(-3r) (masked->0), s1 = sum_c E1 ; v = exp(-4r)
            E1 = dp.tile([128, C], f32, tag="E1")
            s1 = dp.tile([128, 1], f32, tag="s1")
            nc.scalar.activation(E1, r, AF.Exp, scale=-3.0, accum_out=s1)
            v = dp.tile([128, C], f32, tag="v")
            nc.scalar.activation(v, r, AF.Exp, scale=-4.0)

            r1 = dp.tile([128, 1], f32, tag="r1")
            nc.vector.reciprocal(r1, s1)
            r110 = dp.tile([128, 1], f32, tag="r110")
            nc.vector.tensor_scalar_mul(r110, r1, 10.0)

            # wta = v - BIG*(1-m) ;  E2 = exp(10/s1 * wta), s2 = sum_c E2
            wta = dp.tile([128, C], f32, tag="wta")
            nc.vector.tensor_sub(wta, v, psum_mask)
            E2 = dp.tile([128, C], f32, tag="E2")
            s2 = dp.tile([128, 1], f32, tag="s2")
            nc.scalar.activation(E2, wta, AF.Exp, scale=r110, accum_out=s2)

            r2 = dp.tile([128, 1], f32, tag="r2")
            nc.vector.reciprocal(r2, s2)

            # u2 = E2*v ;  yt_j = sum_c (1.5*u2) * w_assoc[:,j]
            u2 = dp.tile([128, C], f32, tag="u2")
            nc.vector.tensor_mul(u2, E2, v)
            yt = dp.tile([128, 2], f32, tag="yt")
            scr0 = dp.tile([128, C], f32, tag="scr0")
            scr1 = dp.tile([128, C], f32, tag="scr1")
            nc.vector.scalar_tensor_tensor(
                out=scr0, in0=u2, scalar=1.5, in1=psum_wa[:, 0, :],
                op0=OP.mult, op1=OP.mult, accum_out=yt[:, 0:1],
            )
            nc.vector.scalar_tensor_tensor(
                out=scr1, in0=u2, scalar=1.5, in1=psum_wa[:, 1, :],
                op0=OP.mult, op1=OP.mult, accum_out=yt[:, 1:2],
            )

            # y = yt / (s1*s2)
            rfin = dp.tile([128, 1], f32, tag="rfin")
            nc.vector.tensor_scalar_mul(rfin, r1, r2)
            y_sb = dp.tile([128, 2], f32, tag="y_sb")
            nc.vector.tensor_scalar_mul(y_sb, yt, rfin)

            nc.sync.dma_start(out=y[:, :], in_=y_sb)

    nc.compile()
    return nc


def _get_nc(matmul_dt_name="float32r"):
    if matmul_dt_name not in _CACHE:
        _CACHE[matmul_dt_name] = _build(matmul_dt_name)
    return _CACHE[matmul_dt_name]


def kernel(inp, w_dist, attn, w_assoc, mask, _trace=False, _tmpdir=None,
           _matmul_dt="float32r"):
    from concourse.bass_utils import run_bass_kernel_spmd

    inp = np.asarray(inp, dtype=np.float32)
    w_dist = np.asarray(w_dist, dtype=np.float32)
    attn = np.asarray(attn, dtype=np.float32)
    w_assoc = np.asarray(w_assoc, dtype=np.float32)
    mask = np.asarray(mask, dtype=np.int32)

    # host-side layout prep (no model FLOPs): transpose / concat / shard
    xT_full = inp.T                                 # [D, B]
    wT = w_dist.T                                   # [D, C]
    smalls = np.concatenate(
        [mask.astype(np.float32), w_assoc.T.reshape(-1).astype(np.float32)]
    ).reshape(1, 3 * C)
    smalls = np.ascontiguousarray(smalls, dtype=np.float32)

    nc = _get_nc(_matmul_dt)

    in_maps = []
    for i in range(N_CORES):
        bigi = np.ascontiguousarray(
            np.concatenate([xT_full[:, i * BS : (i + 1) * BS], wT], axis=1)
        )
        in_maps.append({"big": bigi, "smalls": smalls, "u": attn})

    kw = {}
    if _trace:
        kw["trace"] = True
        if _tmpdir:
            kw["tmpdir"] = _tmpdir
    res = run_bass_kernel_spmd(nc, in_maps, core_ids=list(range(N_CORES)), **kw)
    out = np.concatenate([res.results[i]["y"] for i in range(N_CORES)], axis=0)
    if _trace:
        return out.astype(np.float32), res
    return out.astype(np.float32)
